# revision 76
# baseline (speedup 1.0000x reference)
"""Distributed inverse real SHT on 8 Trainium2 NeuronCores (Bass/Tile).

Math (per reference):
    S[c,k,m]  = sum_l x[c,m,l] * pct[m,k,l]          (Legendre synthesis)
    y[c,k,n]  = irfft_{n=1024}(S, norm='forward')
              = sum_m  Sre[c,k,m]*Fc[m,n] + Sim[c,k,m]*Fs[m,n]
    with Fc[m,n] = w_m cos(2*pi*m*n/N), Fs[m,n] = -w_m sin(2*pi*m*n/N),
    w_0 = 1, w_m = 2 otherwise (verified exactly vs np.fft.irfft).
    pct[m,*,l] = 0 for l < m (triangular), and the m=512 row of pct is
    entirely zero (l < 512 always), so the effective mmax is 512.

Sharding: nlat (k) split across the 8 cores -> 64 output latitudes per
core, no inter-core communication.  Each core streams a packed (l >= m)
slice of pct (fp8 e3m4) and x (fp16), does per-m-pair mixed-precision
matmuls into PSUM, PE-transposes S per 128-m block, applies the DFT
matmul per block, and accumulates the block contributions in an SBUF
fp32 accumulator (so the DFT work overlaps the DMA-bound Legendre
streaming).  The mirrored half of the irfft output is written in
forward order and reversed on the host.
"""

import numpy as np
from contextlib import ExitStack


NLAT, NLON = 512, 1024
LMAX, MMAX = 512, 513
M_E = 512            # effective mmax (m=512 row of pct is identically zero)
B, C = 1, 16
NCORES = 8
KC = NLAT // NCORES  # 64 latitudes per core
PAIRS = M_E // 2     # 256 m-pairs
PW = 128             # pct cols per tile (2m x 64k)
XW = 64              # x cols per tile (2m x 2ri x 16c)
TB = PW + 2 * XW     # 256 bytes per tile column in the merged stream
NBLK = 4             # 128-m blocks


def _plan():
    """One column (128 pct f-cols + 64 x f-cols x 128 rows) per (pair,chunk)
    tile, sorted by K descending within each PSUM bank.

    Returns (bank_ops, slab_widths, total_cols); bank_ops[G] is a list of
    (pair, l0, K, col, base=0) with col the GLOBAL column index."""
    bank_ops = []
    slab_widths = []
    ncols = 0
    for G in range(PAIRS // 8):
        tiles = []
        for t in range(8 * G, 8 * G + 8):
            l0 = 2 * t
            L = LMAX - l0
            nch = (L + 127) // 128
            for c in range(nch):
                tiles.append((t, l0 + 128 * c, min(128, L - 128 * c)))
        tiles.sort(key=lambda x: -x[2])
        ops = [(t, l0, K, ncols + i, 0) for i, (t, l0, K) in enumerate(tiles)]
        bank_ops.append(ops)
        slab_widths.append(len(tiles))
        ncols += len(tiles)
    return bank_ops, slab_widths, ncols


_BANK_OPS, _SLAB_W, NCOLS = _plan()
_SLAB_COL0 = np.cumsum([0] + _SLAB_W)     # global first column of each slab
# Slab DMA grouping: banks are fetched as 2-bank pairs -- bigger
# transfers (0.25-2MB) sit much higher on the DMA efficiency curve and
# halve the sync-engine issue cost.  The very last two banks processed
# (block 0, banks 6 and 7) are fetched singly: bank 6's matmuls then
# overlap bank 7's transfer, shortening the tail's dependency pre-chain
# by ~2us.  (4-bank groups at the START were measured slower -- the
# first consumption waits on a bigger transfer with nothing to hide it.)
_GROUPS = ([[0, 1], [2, 3], [4, 5], [6], [7]] +
           [[2 * i, 2 * i + 1] for i in range(4, 16)])
_BANK2GRP = {}
_GRP_SPAN = []
for _gi, _banks in enumerate(_GROUPS):
    for _g in _banks:
        _BANK2GRP[_g] = _gi
    _GRP_SPAN.append((int(_SLAB_COL0[_banks[0]]),
                      int(sum(_SLAB_W[g] for g in _banks))))

# Even/odd DFT folding: compute E[n'] = sum_m wc*Re and O[n'] = sum_m ws*Im
# for n' in [0,512) plus the y[512] column (folded into O's zero column 0);
# then y[n'] = E+O, y[1024-n'] = E-O (mirror reversal done on the host).
NE = NLON // 2       # 512
FW = NE + NE + 16    # wc | ws | (y512 col + pad)


def build_program():
    from concourse import bacc, bass, masks, mybir, tile

    dt = mybir.dt
    nc = bacc.Bacc("TRN2", target_bir_lowering=False, debug=False,
                   num_devices=NCORES)

    streamp = nc.dram_tensor("streamp", [128, NCOLS * PW], dt.float8e3,
                             kind="ExternalInput")
    streamx = nc.dram_tensor("streamx", [128, NCOLS * XW], dt.float16,
                             kind="ExternalInput")
    fmat = nc.dram_tensor("fmat", [128, NBLK * FW], dt.float8e3,
                          kind="ExternalInput")
    y = nc.dram_tensor("y", [C * KC, NLON], dt.float16, kind="ExternalOutput")

    with tile.TileContext(nc) as tc, ExitStack() as ctx:
        spp = ctx.enter_context(tc.tile_pool(name="streamp", bufs=7))
        spx = ctx.enter_context(tc.tile_pool(name="streamx", bufs=7))
        cp = ctx.enter_context(tc.tile_pool(name="const", bufs=1))
        # bufs=4: a block's DFT-matrix slot otherwise frees only at the END
        # of the next block's banks, and the waiting fsb DMA head-of-line
        # blocks the stream slabs behind it in the FIFO sync queue
        fp = ctx.enter_context(tc.tile_pool(name="fsb", bufs=4))
        ysp = ctx.enter_context(tc.tile_pool(name="ysb", bufs=3))
        flp = ctx.enter_context(tc.tile_pool(name="fold", bufs=2))
        snp = ctx.enter_context(tc.tile_pool(name="snat", bufs=2))
        lhp = ctx.enter_context(tc.tile_pool(name="lhs", bufs=4))
        ps1 = ctx.enter_context(
            tc.tile_pool(name="ps1", bufs=2, space=bass.MemorySpace.PSUM))
        pst = ctx.enter_context(
            tc.tile_pool(name="pst", bufs=2, space=bass.MemorySpace.PSUM))
        ps2 = ctx.enter_context(
            tc.tile_pool(name="ps2", bufs=2, space=bass.MemorySpace.PSUM))

        # fp32 output accumulator: partition = (c2,k64) within strip,
        # free = strip*1024 + n
        acc = cp.tile([128, 8 * NLON], dt.float32)

        slabs = {}

        def get_slab(G):
            """Fetch the slab group containing bank G."""
            p = _BANK2GRP[min(G, PAIRS // 8 - 1)]
            if p not in slabs:
                col0, W = _GRP_SPAN[p]
                stp = spp.tile([128, W * PW], dt.float8e3, tag="slabp")
                stx = spx.tile([128, W * XW], dt.float16, tag="slabx")
                nc.sync.dma_start(
                    out=stp[:],
                    in_=streamp[:, col0 * PW:(col0 + W) * PW])
                nc.sync.dma_start(
                    out=stx[:],
                    in_=streamx[:, col0 * XW:(col0 + W) * XW])
                slabs[p] = (stp, stx, col0)
            return slabs[p]

        # prefetch the first slab groups in processing order (block 3 first)
        get_slab(3 * 8)
        get_slab(3 * 8 + 2)
        get_slab(3 * 8 + 4)

        ident = cp.tile([128, 128], dt.float16)
        masks.make_identity(nc, ident[:])

        deferred = []  # previous block's transpose + DFT work, as thunks

        def make_transp(snat_m):
            """Block's post-stage-1 PE transposes into the stage-2 lhsT
            layout.  Emitted interleaved into the next block's stage-1
            stream so the in-order PE never stalls on the extract chain.
            Returns (thunks, lhs tile handle)."""
            thunks = []
            lhs = lhp.tile([128, 2 * C * KC], dt.float16, tag="lhs")
            lhs_v = lhs[:].rearrange("p (r c k) -> p r c k", r=2, c=C, k=KC)

            def transp(cc):
                pt = pst.tile([128, 128], dt.float16, tag="pt")
                nc.tensor.transpose(pt[:], snat_m[:, :, cc], ident[:])
                # pt cols = ri*64+k -> lhs f = ri*1024 + cc*64 + k
                eng = nc.scalar.copy if cc % 2 else nc.vector.tensor_copy
                eng(lhs_v[:, :, cc, :], pt[:])

            for cc in range(C):
                thunks.append(lambda cc=cc: transp(cc))
            return thunks, lhs

        def make_pair_dft(pi, pairs):
            """DFT matmuls for a PAIR of m-blocks, accumulated in PSUM.
            pi=0: first pair -> copy into the SBUF accumulator.
            pi=1: second pair -> add + E/O fold + output DMA."""
            def dft(s8):
                # E and O accumulate in SEPARATE single-bank PSUM tiles:
                # the fold's u-add then depends only on the two E matmuls
                # and runs under the O matmuls' shadow (tile-granular
                # dependencies), and each bank recycles earlier
                ype = ps2.tile([128, NE], dt.float32, tag="ypE")
                ypo = ps2.tile([128, NE], dt.float32, tag="ypO")
                o_mms = []
                for x, (lhs, fsb) in enumerate(pairs):
                    l0 = lhs[:, s8 * 128:s8 * 128 + 128]          # Re rows
                    l1 = lhs[:, C * KC + s8 * 128:s8 * 128 + C * KC + 128]
                    # E bank: wc matmuls accumulate across the pair
                    nc.tensor.matmul(ype[:], l0, fsb[:, 0:NE],
                                     start=(x == 0), stop=(x == 1))
                    # O bank + y512 column, interleaved per block
                    o_mms.append((l1, fsb, l0))
                for x, (l1, fsb, l0) in enumerate(o_mms):
                    nc.tensor.matmul(ypo[:], l1, fsb[:, NE:2 * NE],
                                     start=(x == 0), stop=False)
                    nc.tensor.matmul(ypo[:, 0:8], l0,
                                     fsb[:, 2 * NE:2 * NE + 8],
                                     start=False, stop=(x == 1))
                base = s8 * NLON
                if pi == 0:
                    nc.vector.tensor_copy(acc[:, base:base + NE], ype[:])
                    nc.vector.tensor_copy(
                        acc[:, base + NE:base + 2 * NE], ypo[:])
                    return
                # fold E/O into the full spectrum and write out, all
                # forward-stride: ysb[1:NE] = E+O is y[1:512];
                # ysb[NE+1+j] = E[1+j]-O[1+j] is y[1023-j], i.e. the
                # mirrored half stored reversed (host flips it).  The
                # pair sums are staged in fp16 temps: halves the DVE
                # PSUM-read op width and puts the final fold (and the
                # GpSimd mirror sub) on the all-16-bit 2x fast path.
                up = flp.tile([128, NE], dt.float16, tag="u")
                vp = flp.tile([128, NE], dt.float16, tag="v")
                nc.vector.tensor_add(
                    up[:], acc[:, base:base + NE], ype[:])
                nc.vector.tensor_add(
                    vp[:], acc[:, base + NE:base + 2 * NE], ypo[:])
                ysb = ysp.tile([128, NLON], dt.float16, tag="ysb")
                nc.vector.tensor_add(
                    ysb[:, 1:NE], up[:, 1:NE], vp[:, 1:NE])
                nc.scalar.copy(ysb[:, 0:1], up[:, 0:1])
                # issue the two y-half DMAs from different engines: the
                # sync engine is idle in the tail (stream fetches done),
                # so its ~600ns issue overlaps ACT's instead of queuing
                # behind the column copies
                nc.sync.dma_start(
                    out=y[s8 * 128:(s8 + 1) * 128, 0:NE],
                    in_=ysb[:, 0:NE])
                nc.scalar.copy(ysb[:, NE:NE + 1], vp[:, 0:1])
                nc.gpsimd.tensor_sub(
                    ysb[:, NE + 1:NLON], up[:, 1:NE], vp[:, 1:NE])
                nc.scalar.dma_start(
                    out=y[s8 * 128:(s8 + 1) * 128, NE:NLON],
                    in_=ysb[:, NE:NLON])

            return [lambda s8=s8: dft(s8) for s8 in range(8)]

        # Process m-blocks smallest-first (block 3 has the shortest pairs,
        # block 0 the longest): each block's deferred transpose+DFT work
        # then hides inside the NEXT block's larger DMA window.  DFT
        # matmuls accumulate block PAIRS in PSUM, halving the DVE
        # PSUM-read traffic of the SBUF accumulator.
        border = list(range(NBLK - 1, -1, -1))
        bank_seq = [b * 8 + g for b in border for g in range(8)]
        seq_pos = {G: i for i, G in enumerate(bank_seq)}
        stash = []

        for bi, b in enumerate(border):
            # DFT matrix slice for this block (prefetched during stage 1)
            fsb = fp.tile([128, FW], dt.float8e3, tag="fsb")
            nc.sync.dma_start(
                out=fsb[:], in_=fmat[:, b * FW:(b + 1) * FW])

            # S^T staging for this 128-m block:
            #   partition = ri*64+k, free = m_loc*16 + c   (fp16)
            snat = snp.tile([128, 128 * C], dt.float16, tag="snat")
            snat_g = snat[:].rearrange("p (g s two c) -> p g s two c",
                                       g=8, s=8, two=2, c=C)
            snat_m = snat[:].rearrange("p (m c) -> p m c", c=C)

            # ---- stage 1: Legendre matmuls, 8 m-pairs per PSUM bank ----
            for g in range(8):
                G = b * 8 + g
                stp, stx, gcol0 = get_slab(G)
                pos = seq_pos[G]
                if pos + 1 < len(bank_seq):
                    get_slab(bank_seq[pos + 1])
                if pos + 2 < len(bank_seq):
                    get_slab(bank_seq[pos + 2])
                # drain ~3 deferred units from the previous block FIRST:
                # their inputs are long ready, so the in-order PE fills the
                # wait for this bank's slab with useful transpose/DFT work
                for _ in range(3):
                    if deferred:
                        deferred.pop(0)()
                pb = ps1.tile([128, 512], dt.float32, tag="pb")
                pb_v = pb[:].rearrange("p (s mj r c) -> p s mj r c",
                                       s=8, mj=2, r=2, c=C)
                ops = _BANK_OPS[G]
                for j, (t, l0, K, col, base) in enumerate(ops):
                    lc = col - gcol0
                    s = t % 8
                    nc.tensor.matmul(
                        pb[:, s * 64:(s + 1) * 64],
                        stp[base:base + K, lc * PW:lc * PW + PW],
                        stx[base:base + K, lc * XW:lc * XW + XW],
                        start=(j == 0), stop=(j == len(ops) - 1),
                        tile_position=(base, 0))
                # extract diagonal (mi==mj) blocks -> snat (cast fp16),
                # split across DVE and ACT
                for mi in range(2):
                    for r in range(2):
                        eng = (nc.vector.tensor_copy if (mi + r) % 2 == 0
                               else nc.scalar.copy)
                        eng(snat_g[r * 64:(r + 1) * 64, g, :, mi, :],
                            pb_v[mi * 64:(mi + 1) * 64, :, mi, r, :])

            while deferred:
                deferred.pop(0)()
            deferred, lhs = make_transp(snat_m)
            stash.append((lhs, fsb))
            if len(stash) == 2:
                dft_thunks = make_pair_dft(bi // 2, stash)
                if bi == NBLK - 1:
                    # tail: nothing left to hide behind, so start each
                    # dft as soon as its two gating transposes (2*s8,
                    # 2*s8+1) have a couple transposes of slack
                    mix = deferred[:4]
                    for s8 in range(8):
                        if 4 + 2 * s8 < len(deferred):
                            mix += deferred[4 + 2 * s8:6 + 2 * s8]
                        mix.append(dft_thunks[s8])
                    deferred = mix
                else:
                    # all 8 pair-A dfts drain in the next block's window
                    # (splitting them 4+4 across two windows was measured
                    # 23us SLOWER -- it breaks the slab/PSUM pipeline)
                    deferred += dft_thunks
                stash = []

        # last block's work has no next block to hide in
        while deferred:
            deferred.pop(0)()

    nc.compile()
    return nc


def _build_fmat():
    import ml_dtypes

    f8 = ml_dtypes.float8_e3m4
    m = np.arange(M_E)
    n2 = np.arange(NE)
    w = np.where(m == 0, 1.0, 2.0)
    ang = 2.0 * np.pi * np.outer(m, n2) / NLON
    wc = (w[:, None] * np.cos(ang)).astype(f8)     # E weights
    ws = (-w[:, None] * np.sin(ang)).astype(f8)    # O weights
    fz = (w * np.where(m % 2 == 0, 1.0, -1.0)).astype(f8)  # y[512]
    fmat = np.zeros((128, NBLK * FW), f8)
    for b in range(NBLK):
        sl = slice(b * 128, (b + 1) * 128)
        fmat[:, b * FW:b * FW + NE] = wc[sl]
        fmat[:, b * FW + NE:b * FW + 2 * NE] = ws[sl]
        fmat[:, b * FW + 2 * NE] = fz[sl]
    return fmat


_ALL_OPS = [op for ops in _BANK_OPS for op in ops]


def _pack_streams(x_re, x_im, pct):
    """Per-core packed streams: pct tiles in fp8 e3m4, x tiles in fp16."""
    import ml_dtypes

    x_re = np.asarray(x_re, np.float32)
    x_im = np.asarray(x_im, np.float32)
    pct = np.asarray(pct, np.float32)

    # x part is core-independent: build once
    xbuf = np.zeros((128, NCOLS * XW), np.float16)
    xv = xbuf.reshape(128, NCOLS, XW)
    for (t, l0, K, col, base) in _ALL_OPS:
        xr = x_re[0, :, l0:l0 + K, 2 * t:2 * t + 2]   # (c, K, 2m)
        xi = x_im[0, :, l0:l0 + K, 2 * t:2 * t + 2]
        xx = np.stack([xr, xi], axis=0)                # (r, c, K, m)
        xv[base:base + K, col, :] = xx.transpose(2, 3, 0, 1).reshape(K, 64)

    pstreams = []
    for core in range(NCORES):
        k0 = core * KC
        pbuf = np.zeros((128, NCOLS * PW), ml_dtypes.float8_e3m4)
        pv = pbuf.reshape(128, NCOLS, PW)
        for (t, l0, K, col, base) in _ALL_OPS:
            blk = pct[2 * t:2 * t + 2, k0:k0 + KC, l0:l0 + K]  # (2m, 64k, K)
            pv[base:base + K, col, :] = \
                blk.transpose(2, 0, 1).reshape(K, 128).astype(
                    ml_dtypes.float8_e3m4)
        pstreams.append(pbuf)
    return pstreams, xbuf


_NC_CACHE = [None]


def _get_program():
    if _NC_CACHE[0] is None:
        _NC_CACHE[0] = build_program()
    return _NC_CACHE[0]


def run(x_re, x_im, pct, nlon=NLON, trace=False, trace_kwargs=None):
    from concourse.bass_utils import run_bass_kernel_spmd

    assert int(nlon) == NLON
    nc = _get_program()
    fmat = _build_fmat()
    pstreams, xbuf = _pack_streams(x_re, x_im, pct)
    in_maps = [{"streamp": pstreams[i], "streamx": xbuf, "fmat": fmat}
               for i in range(NCORES)]
    res = run_bass_kernel_spmd(nc, in_maps, list(range(NCORES)),
                               trace=trace, **(trace_kwargs or {}))
    out = np.empty((B, C, NLAT, NLON), np.float32)
    for core in range(NCORES):
        yc = res.results[core]["y"].astype(np.float32).reshape(C, KC, NLON)
        # device stores the mirrored half of the spectrum in forward
        # order: col NE+1+j holds y[1023-j]
        yc[:, :, NE + 1:] = yc[:, :, NE + 1:][:, :, ::-1]
        out[0, :, core * KC:(core + 1) * KC, :] = yc
    return out, res


def kernel(x_re, x_im, pct, nlon=NLON, **_unused):
    out, _ = run(x_re, x_im, pct, nlon)
    return out


# revision 77
# speedup vs baseline: 1.0161x; 1.0161x over previous
"""Distributed inverse real SHT on 8 Trainium2 NeuronCores (Bass/Tile).

Math (per reference):
    S[c,k,m]  = sum_l x[c,m,l] * pct[m,k,l]          (Legendre synthesis)
    y[c,k,n]  = irfft_{n=1024}(S, norm='forward')
              = sum_m  Sre[c,k,m]*Fc[m,n] + Sim[c,k,m]*Fs[m,n]
    with Fc[m,n] = w_m cos(2*pi*m*n/N), Fs[m,n] = -w_m sin(2*pi*m*n/N),
    w_0 = 1, w_m = 2 otherwise (verified exactly vs np.fft.irfft).
    pct[m,*,l] = 0 for l < m (triangular), and the m=512 row of pct is
    entirely zero (l < 512 always), so the effective mmax is 512.

Sharding: nlat (k) split across the 8 cores -> 64 output latitudes per
core, no inter-core communication.  Each core streams a packed (l >= m)
slice of pct (fp8 e3m4) and x (fp16), does per-m-pair mixed-precision
matmuls into PSUM, PE-transposes S per 128-m block, applies the DFT
matmul per block, and accumulates the block contributions in an SBUF
fp32 accumulator (so the DFT work overlaps the DMA-bound Legendre
streaming).  The mirrored half of the irfft output is written in
forward order and reversed on the host.
"""

import numpy as np
from contextlib import ExitStack


NLAT, NLON = 512, 1024
LMAX, MMAX = 512, 513
M_E = 512            # effective mmax (m=512 row of pct is identically zero)
B, C = 1, 16
NCORES = 8
KC = NLAT // NCORES  # 64 latitudes per core
PAIRS = M_E // 2     # 256 m-pairs
PW = 128             # pct cols per tile (2m x 64k)
XW = 64              # x cols per tile (2m x 2ri x 16c)
TB = PW + 2 * XW     # 256 bytes per tile column in the merged stream
NBLK = 4             # 128-m blocks


def _plan():
    """One column (128 pct f-cols + 64 x f-cols x 128 rows) per (pair,chunk)
    tile, sorted by K descending within each PSUM bank.

    Returns (bank_ops, slab_widths, total_cols); bank_ops[G] is a list of
    (pair, l0, K, col, base=0) with col the GLOBAL column index."""
    bank_ops = []
    slab_widths = []
    ncols = 0
    for G in range(PAIRS // 8):
        tiles = []
        for t in range(8 * G, 8 * G + 8):
            l0 = 2 * t
            L = LMAX - l0
            nch = (L + 127) // 128
            for c in range(nch):
                tiles.append((t, l0 + 128 * c, min(128, L - 128 * c)))
        tiles.sort(key=lambda x: -x[2])
        ops = [(t, l0, K, ncols + i, 0) for i, (t, l0, K) in enumerate(tiles)]
        bank_ops.append(ops)
        slab_widths.append(len(tiles))
        ncols += len(tiles)
    return bank_ops, slab_widths, ncols


_BANK_OPS, _SLAB_W, NCOLS = _plan()
_SLAB_COL0 = np.cumsum([0] + _SLAB_W)     # global first column of each slab
# Slab DMA grouping: banks are fetched as 2-bank pairs -- bigger
# transfers (0.25-2MB) sit much higher on the DMA efficiency curve and
# halve the sync-engine issue cost.  The very last two banks processed
# (block 0, banks 6 and 7) are fetched singly: bank 6's matmuls then
# overlap bank 7's transfer, shortening the tail's dependency pre-chain
# by ~2us.  (4-bank groups at the START were measured slower -- the
# first consumption waits on a bigger transfer with nothing to hide it.)
_GROUPS = ([[0, 1], [2, 3], [4, 5], [6], [7]] +
           [[2 * i, 2 * i + 1] for i in range(4, 16)])
_BANK2GRP = {}
_GRP_SPAN = []
for _gi, _banks in enumerate(_GROUPS):
    for _g in _banks:
        _BANK2GRP[_g] = _gi
    _GRP_SPAN.append((int(_SLAB_COL0[_banks[0]]),
                      int(sum(_SLAB_W[g] for g in _banks))))

# Even/odd DFT folding: compute E[n'] = sum_m wc*Re and O[n'] = sum_m ws*Im
# for n' in [0,512) plus the y[512] column (folded into O's zero column 0);
# then y[n'] = E+O, y[1024-n'] = E-O (mirror reversal done on the host).
NE = NLON // 2       # 512
FW = NE + NE + 16    # wc | ws | (y512 col + pad)


def build_program():
    from concourse import bacc, bass, masks, mybir, tile

    dt = mybir.dt
    nc = bacc.Bacc("TRN2", target_bir_lowering=False, debug=False,
                   num_devices=NCORES)

    streamp = nc.dram_tensor("streamp", [128, NCOLS * PW], dt.float8e3,
                             kind="ExternalInput")
    streamx = nc.dram_tensor("streamx", [128, NCOLS * XW], dt.float16,
                             kind="ExternalInput")
    fmat = nc.dram_tensor("fmat", [128, NBLK * FW], dt.float8e3,
                          kind="ExternalInput")
    y = nc.dram_tensor("y", [C * KC, NLON], dt.float16, kind="ExternalOutput")

    with tile.TileContext(nc) as tc, ExitStack() as ctx:
        spp = ctx.enter_context(tc.tile_pool(name="streamp", bufs=7))
        spx = ctx.enter_context(tc.tile_pool(name="streamx", bufs=7))
        cp = ctx.enter_context(tc.tile_pool(name="const", bufs=1))
        # bufs=4: a block's DFT-matrix slot otherwise frees only at the END
        # of the next block's banks, and the waiting fsb DMA head-of-line
        # blocks the stream slabs behind it in the FIFO sync queue
        fp = ctx.enter_context(tc.tile_pool(name="fsb", bufs=4))
        ysp = ctx.enter_context(tc.tile_pool(name="ysb", bufs=3))
        flp = ctx.enter_context(tc.tile_pool(name="fold", bufs=2))
        snp = ctx.enter_context(tc.tile_pool(name="snat", bufs=2))
        lhp = ctx.enter_context(tc.tile_pool(name="lhs", bufs=4))
        ps1 = ctx.enter_context(
            tc.tile_pool(name="ps1", bufs=2, space=bass.MemorySpace.PSUM))
        pst = ctx.enter_context(
            tc.tile_pool(name="pst", bufs=2, space=bass.MemorySpace.PSUM))
        ps2 = ctx.enter_context(
            tc.tile_pool(name="ps2", bufs=2, space=bass.MemorySpace.PSUM))

        # fp32 output accumulator: partition = (c2,k64) within strip,
        # free = strip*1024 + n
        acc = cp.tile([128, 8 * NLON], dt.float32)

        slabs = {}

        def get_slab(G):
            """Fetch the slab group containing bank G."""
            p = _BANK2GRP[min(G, PAIRS // 8 - 1)]
            if p not in slabs:
                col0, W = _GRP_SPAN[p]
                stp = spp.tile([128, W * PW], dt.float8e3, tag="slabp")
                stx = spx.tile([128, W * XW], dt.float16, tag="slabx")
                nc.sync.dma_start(
                    out=stp[:],
                    in_=streamp[:, col0 * PW:(col0 + W) * PW])
                nc.sync.dma_start(
                    out=stx[:],
                    in_=streamx[:, col0 * XW:(col0 + W) * XW])
                slabs[p] = (stp, stx, col0)
            return slabs[p]

        # prefetch the first slab groups in processing order (block 3 first)
        get_slab(3 * 8)
        get_slab(3 * 8 + 2)
        get_slab(3 * 8 + 4)

        ident = cp.tile([128, 128], dt.float16)
        masks.make_identity(nc, ident[:])

        deferred = []  # previous block's transpose + DFT work, as thunks

        def make_transp(snat_m):
            """Block's post-stage-1 PE transposes into the stage-2 lhsT
            layout.  Emitted interleaved into the next block's stage-1
            stream so the in-order PE never stalls on the extract chain.
            Returns (thunks, lhs tile handle)."""
            thunks = []
            lhs = lhp.tile([128, 2 * C * KC], dt.float16, tag="lhs")
            lhs_v = lhs[:].rearrange("p (r c k) -> p r c k", r=2, c=C, k=KC)

            def transp(cc):
                pt = pst.tile([128, 128], dt.float16, tag="pt")
                nc.tensor.transpose(pt[:], snat_m[:, :, cc], ident[:])
                # pt cols = ri*64+k -> lhs f = ri*1024 + cc*64 + k
                eng = nc.scalar.copy if cc % 2 else nc.vector.tensor_copy
                eng(lhs_v[:, :, cc, :], pt[:])

            for cc in range(C):
                thunks.append(lambda cc=cc: transp(cc))
            return thunks, lhs

        def make_pair_dft(pi, pairs):
            """DFT matmuls for a PAIR of m-blocks, accumulated in PSUM.
            pi=0: first pair -> copy into the SBUF accumulator.
            pi=1: second pair -> add + E/O fold + output DMA."""
            def dft(s8):
                yp = ps2.tile([128, 1024], dt.float32, tag="yp")
                o_mms = []
                for x, (lhs, fsb) in enumerate(pairs):
                    l0 = lhs[:, s8 * 128:s8 * 128 + 128]          # Re rows
                    l1 = lhs[:, C * KC + s8 * 128:s8 * 128 + C * KC + 128]
                    # E bank: wc matmuls accumulate across the pair
                    nc.tensor.matmul(yp[:, 0:NE], l0, fsb[:, 0:NE],
                                     start=(x == 0), stop=(x == 1))
                    # O bank + y512 column, interleaved per block
                    o_mms.append((l1, fsb, l0))
                for x, (l1, fsb, l0) in enumerate(o_mms):
                    nc.tensor.matmul(yp[:, NE:2 * NE], l1, fsb[:, NE:2 * NE],
                                     start=(x == 0), stop=False)
                    nc.tensor.matmul(yp[:, NE:NE + 8], l0,
                                     fsb[:, 2 * NE:2 * NE + 8],
                                     start=False, stop=(x == 1))
                base = s8 * NLON
                if pi == 0:
                    nc.vector.tensor_copy(acc[:, base:base + NLON], yp[:])
                    return
                # fold E/O into the full spectrum and write out, all
                # forward-stride: ysb[1:NE] = E+O is y[1:512];
                # ysb[NE+1+j] = E[1+j]-O[1+j] is y[1023-j], i.e. the
                # mirrored half stored reversed (host flips it).  The
                # pair sums are staged in fp16 temps: halves the DVE
                # PSUM-read op width and puts the final fold (and the
                # GpSimd mirror sub) on the all-16-bit 2x fast path.
                up = flp.tile([128, NE], dt.float16, tag="u")
                vp = flp.tile([128, NE], dt.float16, tag="v")
                nc.vector.tensor_add(
                    up[:], acc[:, base:base + NE], yp[:, 0:NE])
                nc.vector.tensor_add(
                    vp[:], acc[:, base + NE:base + 2 * NE], yp[:, NE:2 * NE])
                ysb = ysp.tile([128, NLON], dt.float16, tag="ysb")
                nc.vector.tensor_add(
                    ysb[:, 1:NE], up[:, 1:NE], vp[:, 1:NE])
                nc.scalar.copy(ysb[:, 0:1], up[:, 0:1])
                # issue the two y-half DMAs from different engines: the
                # sync engine is idle in the tail (stream fetches done),
                # so its ~600ns issue overlaps ACT's instead of queuing
                # behind the column copies
                nc.sync.dma_start(
                    out=y[s8 * 128:(s8 + 1) * 128, 0:NE],
                    in_=ysb[:, 0:NE])
                nc.scalar.copy(ysb[:, NE:NE + 1], vp[:, 0:1])
                nc.gpsimd.tensor_sub(
                    ysb[:, NE + 1:NLON], up[:, 1:NE], vp[:, 1:NE])
                nc.scalar.dma_start(
                    out=y[s8 * 128:(s8 + 1) * 128, NE:NLON],
                    in_=ysb[:, NE:NLON])

            return [lambda s8=s8: dft(s8) for s8 in range(8)]

        # Process m-blocks smallest-first (block 3 has the shortest pairs,
        # block 0 the longest): each block's deferred transpose+DFT work
        # then hides inside the NEXT block's larger DMA window.  DFT
        # matmuls accumulate block PAIRS in PSUM, halving the DVE
        # PSUM-read traffic of the SBUF accumulator.
        border = list(range(NBLK - 1, -1, -1))
        bank_seq = [b * 8 + g for b in border for g in range(8)]
        seq_pos = {G: i for i, G in enumerate(bank_seq)}
        stash = []

        for bi, b in enumerate(border):
            # DFT matrix slice for this block (prefetched during stage 1)
            fsb = fp.tile([128, FW], dt.float8e3, tag="fsb")
            nc.sync.dma_start(
                out=fsb[:], in_=fmat[:, b * FW:(b + 1) * FW])

            # S^T staging for this 128-m block:
            #   partition = ri*64+k, free = m_loc*16 + c   (fp16)
            snat = snp.tile([128, 128 * C], dt.float16, tag="snat")
            snat_g = snat[:].rearrange("p (g s two c) -> p g s two c",
                                       g=8, s=8, two=2, c=C)
            snat_m = snat[:].rearrange("p (m c) -> p m c", c=C)

            # ---- stage 1: Legendre matmuls, 8 m-pairs per PSUM bank ----
            for g in range(8):
                G = b * 8 + g
                stp, stx, gcol0 = get_slab(G)
                pos = seq_pos[G]
                if pos + 1 < len(bank_seq):
                    get_slab(bank_seq[pos + 1])
                if pos + 2 < len(bank_seq):
                    get_slab(bank_seq[pos + 2])
                # drain ~3 deferred units from the previous block FIRST:
                # their inputs are long ready, so the in-order PE fills the
                # wait for this bank's slab with useful transpose/DFT work
                for _ in range(3):
                    if deferred:
                        deferred.pop(0)()
                pb = ps1.tile([128, 512], dt.float32, tag="pb")
                pb_v = pb[:].rearrange("p (s mj r c) -> p s mj r c",
                                       s=8, mj=2, r=2, c=C)
                ops = _BANK_OPS[G]
                for j, (t, l0, K, col, base) in enumerate(ops):
                    lc = col - gcol0
                    s = t % 8
                    nc.tensor.matmul(
                        pb[:, s * 64:(s + 1) * 64],
                        stp[base:base + K, lc * PW:lc * PW + PW],
                        stx[base:base + K, lc * XW:lc * XW + XW],
                        start=(j == 0), stop=(j == len(ops) - 1),
                        tile_position=(base, 0))
                # extract diagonal (mi==mj) blocks -> snat (cast fp16),
                # split across DVE and ACT
                for mi in range(2):
                    for r in range(2):
                        eng = (nc.vector.tensor_copy if (mi + r) % 2 == 0
                               else nc.scalar.copy)
                        eng(snat_g[r * 64:(r + 1) * 64, g, :, mi, :],
                            pb_v[mi * 64:(mi + 1) * 64, :, mi, r, :])

            while deferred:
                deferred.pop(0)()
            deferred, lhs = make_transp(snat_m)
            stash.append((lhs, fsb))
            if len(stash) == 2:
                dft_thunks = make_pair_dft(bi // 2, stash)
                if bi == NBLK - 1:
                    # tail: nothing left to hide behind, so start each
                    # dft as soon as its two gating transposes (2*s8,
                    # 2*s8+1) have a couple transposes of slack
                    mix = deferred[:4]
                    for s8 in range(8):
                        if 4 + 2 * s8 < len(deferred):
                            mix += deferred[4 + 2 * s8:6 + 2 * s8]
                        mix.append(dft_thunks[s8])
                    deferred = mix
                else:
                    # all 8 pair-A dfts drain in the next block's window
                    # (splitting them 4+4 across two windows was measured
                    # 23us SLOWER -- it breaks the slab/PSUM pipeline)
                    deferred += dft_thunks
                stash = []

        # last block's work has no next block to hide in
        while deferred:
            deferred.pop(0)()

    nc.compile()
    return nc


def _build_fmat():
    import ml_dtypes

    f8 = ml_dtypes.float8_e3m4
    m = np.arange(M_E)
    n2 = np.arange(NE)
    w = np.where(m == 0, 1.0, 2.0)
    ang = 2.0 * np.pi * np.outer(m, n2) / NLON
    wc = (w[:, None] * np.cos(ang)).astype(f8)     # E weights
    ws = (-w[:, None] * np.sin(ang)).astype(f8)    # O weights
    fz = (w * np.where(m % 2 == 0, 1.0, -1.0)).astype(f8)  # y[512]
    fmat = np.zeros((128, NBLK * FW), f8)
    for b in range(NBLK):
        sl = slice(b * 128, (b + 1) * 128)
        fmat[:, b * FW:b * FW + NE] = wc[sl]
        fmat[:, b * FW + NE:b * FW + 2 * NE] = ws[sl]
        fmat[:, b * FW + 2 * NE] = fz[sl]
    return fmat


_ALL_OPS = [op for ops in _BANK_OPS for op in ops]


def _pack_streams(x_re, x_im, pct):
    """Per-core packed streams: pct tiles in fp8 e3m4, x tiles in fp16."""
    import ml_dtypes

    x_re = np.asarray(x_re, np.float32)
    x_im = np.asarray(x_im, np.float32)
    pct = np.asarray(pct, np.float32)

    # x part is core-independent: build once
    xbuf = np.zeros((128, NCOLS * XW), np.float16)
    xv = xbuf.reshape(128, NCOLS, XW)
    for (t, l0, K, col, base) in _ALL_OPS:
        xr = x_re[0, :, l0:l0 + K, 2 * t:2 * t + 2]   # (c, K, 2m)
        xi = x_im[0, :, l0:l0 + K, 2 * t:2 * t + 2]
        xx = np.stack([xr, xi], axis=0)                # (r, c, K, m)
        xv[base:base + K, col, :] = xx.transpose(2, 3, 0, 1).reshape(K, 64)

    pstreams = []
    for core in range(NCORES):
        k0 = core * KC
        pbuf = np.zeros((128, NCOLS * PW), ml_dtypes.float8_e3m4)
        pv = pbuf.reshape(128, NCOLS, PW)
        for (t, l0, K, col, base) in _ALL_OPS:
            blk = pct[2 * t:2 * t + 2, k0:k0 + KC, l0:l0 + K]  # (2m, 64k, K)
            pv[base:base + K, col, :] = \
                blk.transpose(2, 0, 1).reshape(K, 128).astype(
                    ml_dtypes.float8_e3m4)
        pstreams.append(pbuf)
    return pstreams, xbuf


_NC_CACHE = [None]


def _get_program():
    if _NC_CACHE[0] is None:
        _NC_CACHE[0] = build_program()
    return _NC_CACHE[0]


def run(x_re, x_im, pct, nlon=NLON, trace=False, trace_kwargs=None):
    from concourse.bass_utils import run_bass_kernel_spmd

    assert int(nlon) == NLON
    nc = _get_program()
    fmat = _build_fmat()
    pstreams, xbuf = _pack_streams(x_re, x_im, pct)
    in_maps = [{"streamp": pstreams[i], "streamx": xbuf, "fmat": fmat}
               for i in range(NCORES)]
    res = run_bass_kernel_spmd(nc, in_maps, list(range(NCORES)),
                               trace=trace, **(trace_kwargs or {}))
    out = np.empty((B, C, NLAT, NLON), np.float32)
    for core in range(NCORES):
        yc = res.results[core]["y"].astype(np.float32).reshape(C, KC, NLON)
        # device stores the mirrored half of the spectrum in forward
        # order: col NE+1+j holds y[1023-j]
        yc[:, :, NE + 1:] = yc[:, :, NE + 1:][:, :, ::-1]
        out[0, :, core * KC:(core + 1) * KC, :] = yc
    return out, res


def kernel(x_re, x_im, pct, nlon=NLON, **_unused):
    out, _ = run(x_re, x_im, pct, nlon)
    return out
